# revision 15
# baseline (speedup 1.0000x reference)
"""GCN (2-layer GraphConv + classifier) on 8 Trainium2 NeuronCores.

v3: pipelined halves.
- Nodes -> slots via a 2D load-balancing permutation (per-(tile, src-half)
  edge buckets near-equal, minimizing 128-pad + max-over-cores).
- h published in two chunked AllGathers (per-core row halves, tiles 0-23 /
  24-48) so the collective overlaps phase-1 compute and the layer-1 gathers;
  same for h1s and the layer-2 gathers.
- Aggregation per half-sweep: SWDGE dma_gather (768B rows, 4 queues) + one-hot
  M matmuls; the lo-half partial parks in an SBUF bf16 accumulator and is
  re-injected in the hi sweep with an identity matmul; bias enters as a rank-1
  matmul; finalize is one scalar-engine op via relu(s*x) = s*relu(x).
- Layer 2 aggregates h1s = relu(agg1*inv_d+b1)*inv_s (384 wide) and folds
  (W2 @ Wc) after aggregation, so both layers share one gather structure.
- One-hot M tiles generated on-chip, one broadcast is_equal per gather call.
"""
import os
import sys

sys.path.insert(0, "/opt/trn_rl_repo")

import numpy as np
import ml_dtypes

import concourse.bacc as bacc
import concourse.bass as bass
import concourse.mybir as mybir
import concourse.tile as tile
from concourse import library_config
from concourse.masks import make_identity

NCORES = 8
P = 128
N_NODES = 50000
N_EDGES = 400000
NP_PAD = 50176            # 8 * 6272
R = NP_PAD // NCORES      # 6272 rows per core
RT = R // P               # 49 row tiles per core
LOT = 24                  # lo-half tiles per core (0..23)
HIT = RT - LOT            # hi-half tiles per core (24..48)
LR = LOT * P              # 3072 lo rows per core
HR = HIT * P              # 3200 hi rows per core
IN_F = 1433
KP = 1536                 # padded contraction (12 * 128)
KC = KP // P              # 12 k-chunks
HID = 384
N_CLS = 7
GROUP_TILES = 2           # dst tiles per PSUM group (bank bound)
SG = 4                    # PSUM groups per gather call (supergroup)
NSWQ = 4                  # SWDGE queues for gathers
HP8 = 512                 # fp8 h-table row width (512B rows for dma_gather)

bf16 = ml_dtypes.bfloat16


def _balance_nodes(edge_src, edge_dst):
    """Permute nodes -> slots: per-core row-half by out-degree, 128-node dst
    bins balancing (in_lo, in_hi) jointly, bins paired across cores by size."""
    src = edge_src.astype(np.int64)
    dst = edge_dst.astype(np.int64)
    deg_out = np.bincount(src, minlength=NP_PAD).astype(np.int64)

    caps = (NCORES * LR, NCORES * HR)
    order = np.argsort(-deg_out, kind="stable")
    half_of = np.zeros(NP_PAD, np.int8)
    sums = [0, 0]
    counts = [0, 0]
    for n in order:
        h = 0 if (sums[0] <= sums[1] and counts[0] < caps[0]) or counts[1] >= caps[1] else 1
        half_of[n] = h
        sums[h] += deg_out[n]
        counts[h] += 1

    in_lo = np.bincount(dst[half_of[src] == 0], minlength=NP_PAD).astype(np.int64)
    in_hi = np.bincount(dst[half_of[src] == 1], minlength=NP_PAD).astype(np.int64)

    bins_of_half = {}
    for h, nb in ((0, NCORES * LOT), (1, NCORES * HIT)):
        nodes = np.nonzero(half_of == h)[0]
        keys = np.maximum(in_lo[nodes], in_hi[nodes])
        nodes = nodes[np.argsort(-keys, kind="stable")]
        bsum = np.zeros((nb, 2), np.int64)
        bcnt = np.zeros(nb, np.int64)
        assign = np.empty(len(nodes), np.int64)
        big = np.iinfo(np.int64).max
        for i in range(len(nodes)):
            n = nodes[i]
            cost = np.maximum(bsum[:, 0] + in_lo[n], bsum[:, 1] + in_hi[n])
            cost = np.where(bcnt < P, cost, big)
            b = int(np.argmin(cost))
            assign[i] = b
            bsum[b, 0] += in_lo[n]
            bsum[b, 1] += in_hi[n]
            bcnt[b] += 1
        bins_of_half[h] = ([nodes[assign == b] for b in range(nb)], bsum)

    perm = np.empty(NP_PAD, np.int64)
    for h, tiles in ((0, range(0, LOT)), (1, range(LOT, RT))):
        bins, bsum = bins_of_half[h]
        rank = np.argsort(-(bsum[:, 0] * 100000 + bsum[:, 1]), kind="stable")
        k = 0
        for t in tiles:
            for c in range(NCORES):
                nodes = bins[rank[k]]
                k += 1
                assert len(nodes) == P
                perm[nodes] = c * R + t * P + np.arange(P)
    return perm


def _build_edge_plan(edge_src, edge_dst):
    src0 = edge_src.astype(np.int64)
    dst0 = edge_dst.astype(np.int64)
    perm = _balance_nodes(src0, dst0)
    src = perm[src0]
    dst = perm[dst0]

    core = dst // R
    w_d = dst % R
    t_all = w_d // P
    p_all = w_d % P
    c_s = src // R
    w_s = src % R
    half_all = (w_s >= LR).astype(np.int64)
    # index into the lo/hi AllGather tables
    src_rel = np.where(half_all == 0, c_s * LR + w_s, c_s * HR + (w_s - LR))

    counts = np.zeros((NCORES, RT, 2), np.int64)
    np.add.at(counts, (core, t_all, half_all), 1)
    chunks = np.maximum(np.ceil(counts / P).astype(np.int64).max(axis=0), 1)
    chunks_lo = chunks[:, 0]
    chunks_hi = chunks[:, 1]

    groups = [list(range(g, min(g + GROUP_TILES, RT)))
              for g in range(0, RT, GROUP_TILES)]

    # chunk layout: lo block (per group, tiles in order), then hi block
    lo_off = np.zeros(RT, np.int64)
    hi_off = np.zeros(RT, np.int64)
    lo_base = []
    hi_base = []
    ofs = 0
    for tiles in groups:
        lo_base.append(ofs)
        for t in tiles:
            lo_off[t] = ofs
            ofs += chunks_lo[t]
    clo = ofs
    for tiles in groups:
        hi_base.append(ofs)
        for t in tiles:
            hi_off[t] = ofs
            ofs += chunks_hi[t]
    c_tot = ofs

    order = np.lexsort((src_rel, half_all, t_all, core))
    idx_all = np.zeros((NCORES, c_tot * P), np.int64)
    pcol_all = np.full((NCORES, c_tot * P), -1.0, np.float32)

    srt_core = core[order]
    srt_t = t_all[order]
    srt_half = half_all[order]
    srt_src = src_rel[order]
    srt_p = p_all[order]

    core_starts = np.searchsorted(srt_core, np.arange(NCORES + 1))
    for c in range(NCORES):
        s, e = core_starts[c], core_starts[c + 1]
        tt = srt_t[s:e]
        hh = srt_half[s:e]
        key = tt * 2 + hh
        if len(key):
            new_run = np.concatenate([[True], key[1:] != key[:-1]])
            run_ids = np.cumsum(new_run) - 1
            first_pos = np.nonzero(new_run)[0]
            run_start = first_pos[run_ids]
            pos_in_run = np.arange(len(key)) - run_start
            base = np.where(hh == 0, lo_off[tt], hi_off[tt]) * P
            gpos = base + pos_in_run
            idx_all[c][gpos] = srt_src[s:e]
            pcol_all[c][gpos] = srt_p[s:e]

    pcol = np.ascontiguousarray(
        pcol_all.reshape(NCORES, c_tot, P).transpose(0, 2, 1)).astype(np.float32)

    idx_wrapped = np.zeros((NCORES, P, c_tot * P // 16), np.int16)
    for c in range(NCORES):
        w = idx_all[c].astype(np.int16).reshape(-1, 16).T
        idx_wrapped[c] = np.tile(w, (8, 1))

    return dict(
        chunks_lo=chunks_lo, chunks_hi=chunks_hi, groups=groups,
        lo_base=lo_base, hi_base=hi_base, c_tot=c_tot,
        pcol=pcol, idx_wrapped=idx_wrapped, perm=perm,
    )


ALL_PHASES = frozenset(["p1", "ag1", "g1", "mm1", "ag2", "g2", "mm2", "fin"])


def _build_nc(plan, repeat=1, phases=None, swq=NSWQ):
    on = ALL_PHASES if phases is None else frozenset(phases)
    chunks_lo = plan["chunks_lo"]
    chunks_hi = plan["chunks_hi"]
    groups = plan["groups"]
    lo_base = plan["lo_base"]
    hi_base = plan["hi_base"]
    c_tot = plan["c_tot"]

    nc = bacc.Bacc("TRN2", target_bir_lowering=False, debug=False,
                   num_devices=NCORES, num_swdge_queues=swq)
    dt = mybir.dt

    # ---- I/O ----
    xT = nc.dram_tensor("xT", [RT, P, KC * P], dt.bfloat16, kind="ExternalInput")
    w1 = nc.dram_tensor("w1", [P, KC * HID], dt.bfloat16, kind="ExternalInput")
    w2c = nc.dram_tensor("w2c", [P, 3 * 8], dt.bfloat16, kind="ExternalInput")
    b1r = nc.dram_tensor("b1r", [1, HID], dt.bfloat16, kind="ExternalInput")
    bcr = nc.dram_tensor("bcr", [1, 8], dt.bfloat16, kind="ExternalInput")
    binv = nc.dram_tensor("binv", [1, R], dt.bfloat16, kind="ExternalInput")
    inv_s_t = nc.dram_tensor("inv_s_t", [P, RT], dt.float32, kind="ExternalInput")
    inv_d_t = nc.dram_tensor("inv_d_t", [P, RT], dt.float32, kind="ExternalInput")
    sc3_in = nc.dram_tensor("sc3", [P, RT], dt.float32, kind="ExternalInput")
    pcol_in = nc.dram_tensor("pcol", [P, c_tot], dt.float32, kind="ExternalInput")
    iota_in = nc.dram_tensor("iota", [P, 1, P], dt.float32, kind="ExternalInput")
    idxs = nc.dram_tensor("idxs", [P, c_tot * P // 16], dt.int16, kind="ExternalInput")
    out = nc.dram_tensor("out", [P, RT * N_CLS], dt.float32, kind="ExternalOutput")

    # ---- internal DRAM ----
    h_c_lo = nc.dram_tensor("h_c_lo", [LR, HID], dt.bfloat16)
    h_c_hi = nc.dram_tensor("h_c_hi", [HR, HID], dt.bfloat16)
    h_lo_tab = nc.dram_tensor("h_lo_tab", [NCORES * LR, HID], dt.bfloat16, addr_space="Shared")
    h_hi_tab = nc.dram_tensor("h_hi_tab", [NCORES * HR, HID], dt.bfloat16, addr_space="Shared")
    h1_c_lo = nc.dram_tensor("h1_c_lo", [LR, HID], dt.bfloat16)
    h1_c_hi = nc.dram_tensor("h1_c_hi", [HR, HID], dt.bfloat16)
    h1_lo_tab = nc.dram_tensor("h1_lo_tab", [NCORES * LR, HID], dt.bfloat16, addr_space="Shared")
    h1_hi_tab = nc.dram_tensor("h1_hi_tab", [NCORES * HR, HID], dt.bfloat16, addr_space="Shared")

    rg = [list(range(NCORES))]

    with tile.TileContext(nc) as tc:
        with (
            tc.tile_pool(name="const", bufs=1) as const,
            tc.tile_pool(name="xload", bufs=3) as xload,
            tc.tile_pool(name="hout", bufs=3) as hout,
            tc.tile_pool(name="gbuf", bufs=2) as gbuf,
            tc.tile_pool(name="mgen", bufs=2) as mgen,
            tc.tile_pool(name="work", bufs=4) as work,
            tc.tile_pool(name="accp", bufs=1) as accp,
            tc.tile_pool(name="psA", bufs=2, space="PSUM") as psA,
            tc.tile_pool(name="psB", bufs=2, space="PSUM") as psB,
        ):
            nc.gpsimd.load_library(library_config.mlp)
            qctr = [0]

            def nextq():
                q = qctr[0] % swq
                qctr[0] += 1
                return q

            w1_t = const.tile([P, KC * HID], dt.bfloat16)
            nc.sync.dma_start(out=w1_t[:], in_=w1[:])
            w2c_t = const.tile([P, 3 * 8], dt.bfloat16)
            nc.sync.dma_start(out=w2c_t[:], in_=w2c[:])
            b1_t = const.tile([1, HID], dt.bfloat16)
            nc.sync.dma_start(out=b1_t[:], in_=b1r[:])
            bc_t = const.tile([1, 8], dt.bfloat16)
            nc.sync.dma_start(out=bc_t[:], in_=bcr[:])
            binv_t = const.tile([1, R], dt.bfloat16)
            nc.sync.dma_start(out=binv_t[:], in_=binv[:])
            invs_t = const.tile([P, RT], dt.float32)
            nc.sync.dma_start(out=invs_t[:], in_=inv_s_t[:])
            invd_t = const.tile([P, RT], dt.float32)
            nc.sync.dma_start(out=invd_t[:], in_=inv_d_t[:])
            sc3_t = const.tile([P, RT], dt.float32)
            nc.sync.dma_start(out=sc3_t[:], in_=sc3_in[:])
            pcol_t = const.tile([P, c_tot, 1], dt.float32)
            nc.sync.dma_start(out=pcol_t[:, :, 0], in_=pcol_in[:])
            iota_t = const.tile([P, 1, P], dt.float32)
            nc.sync.dma_start(out=iota_t[:], in_=iota_in[:])
            idx_t = const.tile([P, c_tot * P // 16], dt.int16)
            nc.sync.dma_start(out=idx_t[:], in_=idxs[:])
            ident = const.tile([P, P], dt.bfloat16)
            make_identity(nc, ident[:])
            ones_t = const.tile([1, P], dt.bfloat16)
            nc.vector.memset(ones_t[:], 1.0)
            dummy = const.tile([P, HID], dt.bfloat16)
            nc.vector.memset(dummy[:], 0.0)

            out_t = const.tile([P, RT * N_CLS], dt.float32)
            acc_t = accp.tile([P, RT, HID], dt.bfloat16)

            def sweep(layer, half, tab):
                """One half-sweep of gather+aggregate over all groups.

                half==0: partials -> acc_t.  half==1: finish (acc re-injected
                via identity matmul), finalize per tile."""
                gk = "g1" if layer == 1 else "g2"
                mmk = "mm1" if layer == 1 else "mm2"
                gdt = dt.bfloat16
                gw = HID
                chunks = chunks_lo if half == 0 else chunks_hi
                cbases = lo_base if half == 0 else hi_base
                for sg in range(0, len(groups), SG):
                    sgroups = groups[sg:sg + SG]
                    cbase = cbases[sg]
                    n_g = int(sum(chunks[t] for tiles in sgroups for t in tiles))
                    g_t = gbuf.tile([P, n_g, gw], gdt, tag="g")
                    if gk in on:
                        nc.gpsimd.dma_gather(
                            out_ap=g_t[:], in_ap=tab,
                            idxs_ap=idx_t[:, cbase * 8:(cbase + n_g) * 8],
                            num_idxs=n_g * P, num_idxs_reg=n_g * P,
                            elem_size=gw,
                            single_packet=False, queue_num=nextq(),
                        )
                    m_t = mgen.tile([P, n_g, P], dt.bfloat16, tag="m")
                    if mmk in on:
                        nc.vector.tensor_tensor(
                            out=m_t[:],
                            in0=pcol_t[:, cbase:cbase + n_g, :].to_broadcast([P, n_g, P]),
                            in1=iota_t[:].to_broadcast([P, n_g, P]),
                            op=mybir.AluOpType.is_equal,
                        )
                    sg_tiles = [t for tiles in sgroups for t in tiles]
                    pos = 0
                    for ti, t in enumerate(sg_tiles if mmk in on else []):
                        ncn = int(chunks[t])
                        ps = psA.tile([P, HID], dt.float32, space="PSUM",
                                      tag=f"ps{ti % GROUP_TILES}")
                        for j in range(ncn):
                            nc.tensor.matmul(
                                out=ps[:], lhsT=m_t[:, pos + j, :],
                                rhs=g_t[:, pos + j, 0:HID] if gk in on else dummy[:],
                                start=(j == 0),
                                stop=(half == 0 and j == ncn - 1),
                            )
                        pos += ncn
                        if half == 0:
                            nc.scalar.activation(
                                out=acc_t[:, t, :], in_=ps[:],
                                func=mybir.ActivationFunctionType.Copy,
                            )
                            continue
                        # hi sweep: re-inject lo partial
                        nc.tensor.matmul(
                            out=ps[:], lhsT=ident[:], rhs=acc_t[:, t, :],
                            start=False, stop=(layer == 2),
                        )
                        if layer == 1:
                            # + outer(1/inv_d, b1)
                            nc.tensor.matmul(
                                out=ps[:],
                                lhsT=binv_t[0:1, t * P:(t + 1) * P],
                                rhs=b1_t[:],
                                start=False, stop=True,
                            )
                            h1t = work.tile([P, HID], dt.bfloat16, tag="h1t")
                            nc.scalar.activation(
                                out=h1t[:], in_=ps[:],
                                func=mybir.ActivationFunctionType.Relu,
                                scale=sc3_t[:, t:t + 1],
                            )
                            if t < LOT:
                                nc.sync.dma_start(
                                    out=h1_c_lo[t * P:(t + 1) * P, :], in_=h1t[:])
                            else:
                                nc.sync.dma_start(
                                    out=h1_c_hi[(t - LOT) * P:(t - LOT + 1) * P, :],
                                    in_=h1t[:])
                        else:
                            sc = work.tile([P, HID], dt.bfloat16, tag="sc")
                            nc.scalar.activation(
                                out=sc[:], in_=ps[:],
                                func=mybir.ActivationFunctionType.Copy,
                                scale=invd_t[:, t:t + 1],
                            )
                            aggT = work.tile([P, HID], dt.bfloat16, tag="aggT")
                            for k in range(3):
                                pst = psB.tile([P, P], dt.bfloat16, space="PSUM",
                                               tag="pst")
                                nc.tensor.transpose(
                                    out=pst[:], in_=sc[:, k * P:(k + 1) * P],
                                    identity=ident[:],
                                )
                                nc.scalar.activation(
                                    out=aggT[:, k * P:(k + 1) * P], in_=pst[:],
                                    func=mybir.ActivationFunctionType.Copy,
                                )
                            psz = psB.tile([P, 8], dt.float32, space="PSUM",
                                           tag="psz")
                            for k in range(3):
                                nc.tensor.matmul(
                                    out=psz[:],
                                    lhsT=aggT[:, k * P:(k + 1) * P],
                                    rhs=w2c_t[:, k * 8:(k + 1) * 8],
                                    start=(k == 0), stop=False,
                                )
                            nc.tensor.matmul(
                                out=psz[:], lhsT=ones_t[:], rhs=bc_t[:],
                                start=False, stop=True,
                            )
                            nc.scalar.activation(
                                out=out_t[:, t * N_CLS:(t + 1) * N_CLS],
                                in_=psz[:, 0:N_CLS],
                                func=mybir.ActivationFunctionType.Copy,
                            )

            for _rep in range(repeat):
                # ---- Phase 1: h = (x @ W1) * inv_s ----
                if "p1" in on:
                    for r in range(RT):
                        xt = xload.tile([P, KC * P], dt.bfloat16)
                        nc.sync.dma_start(out=xt[:], in_=xT[r])
                        ps = psA.tile([P, HID], dt.float32, space="PSUM", tag="ps0")
                        for k in range(KC):
                            nc.tensor.matmul(
                                out=ps[:],
                                lhsT=xt[:, k * P:(k + 1) * P],
                                rhs=w1_t[:, k * HID:(k + 1) * HID],
                                start=(k == 0),
                                stop=(k == KC - 1),
                            )
                        ht = hout.tile([P, HID], dt.bfloat16)
                        nc.scalar.activation(
                            out=ht[:], in_=ps[:],
                            func=mybir.ActivationFunctionType.Copy,
                            scale=invs_t[:, r:r + 1],
                        )
                        if r < LOT:
                            nc.sync.dma_start(
                                out=h_c_lo[r * P:(r + 1) * P, :], in_=ht[:])
                        else:
                            nc.sync.dma_start(
                                out=h_c_hi[(r - LOT) * P:(r - LOT + 1) * P, :],
                                in_=ht[:])

                # ---- publish h halves (chunked AllGather) ----
                if "ag1" in on:
                    nc.gpsimd.collective_compute(
                        "AllGather", mybir.AluOpType.bypass, replica_groups=rg,
                        ins=[h_c_lo[:]], outs=[h_lo_tab[:]],
                    )
                    nc.gpsimd.collective_compute(
                        "AllGather", mybir.AluOpType.bypass, replica_groups=rg,
                        ins=[h_c_hi[:]], outs=[h_hi_tab[:]],
                    )

                # ---- Phase 3: layer-1 aggregation -> h1s ----
                sweep(1, 0, h_lo_tab[:])
                sweep(1, 1, h_hi_tab[:])

                # ---- publish h1s halves ----
                if "ag2" in on:
                    nc.gpsimd.collective_compute(
                        "AllGather", mybir.AluOpType.bypass, replica_groups=rg,
                        ins=[h1_c_lo[:]], outs=[h1_lo_tab[:]],
                    )
                    nc.gpsimd.collective_compute(
                        "AllGather", mybir.AluOpType.bypass, replica_groups=rg,
                        ins=[h1_c_hi[:]], outs=[h1_hi_tab[:]],
                    )

                # ---- Phase 5: layer-2 aggregation -> logits ----
                sweep(2, 0, h1_lo_tab[:])
                sweep(2, 1, h1_hi_tab[:])

                if "fin" in on:
                    nc.sync.dma_start(out=out[:], in_=out_t[:])

    nc.compile()
    return nc


def _prepare(features, edge_src, edge_dst, W1, b1, W2, b2, Wc, bc):
    deg_out = np.bincount(edge_src, minlength=N_NODES).astype(np.float32)
    deg_in = np.bincount(edge_dst, minlength=N_NODES).astype(np.float32)
    inv_s = 1.0 / np.sqrt(np.maximum(deg_out, 1.0))
    inv_d = 1.0 / np.sqrt(np.maximum(deg_in, 1.0))
    inv_s = np.concatenate([inv_s, np.ones(NP_PAD - N_NODES, np.float32)])
    inv_d = np.concatenate([inv_d, np.ones(NP_PAD - N_NODES, np.float32)])

    plan = _build_edge_plan(edge_src, edge_dst)
    perm = plan["perm"]

    W1p = np.zeros((KP, HID), np.float32)
    W1p[:IN_F] = W1
    W1p = W1p.astype(bf16)
    w1_sw = np.concatenate([W1p[k * P:(k + 1) * P] for k in range(KC)], axis=1)
    W2c = (W2.astype(np.float32) @ Wc.astype(np.float32))
    W2cp = np.zeros((HID, 8), np.float32)
    W2cp[:, :N_CLS] = W2c
    W2cp16 = W2cp.astype(bf16)
    w2c_sw = np.concatenate([W2cp16[k * P:(k + 1) * P] for k in range(3)], axis=1)
    bcp = (b2.astype(np.float32) @ Wc.astype(np.float32) + bc).astype(np.float32)
    bc_row = np.zeros((1, 8), np.float32)
    bc_row[0, :N_CLS] = bcp

    # slot-ordered node data (slot s holds node n with perm[n] == s)
    xpad = np.zeros((NP_PAD, KP), bf16)
    xpad[perm[:N_NODES], :IN_F] = features.astype(bf16)
    inv_s_slot = np.ones(NP_PAD, np.float32)
    inv_d_slot = np.ones(NP_PAD, np.float32)
    inv_s_slot[perm] = inv_s
    inv_d_slot[perm] = inv_d
    sc3_slot = inv_s_slot * inv_d_slot
    binv_slot = 1.0 / inv_d_slot

    iota128 = np.arange(P, dtype=np.float32)[None, None, :] * np.ones(
        (P, 1, 1), np.float32)

    in_maps = []
    for c in range(NCORES):
        sl = slice(c * R, (c + 1) * R)
        xt = np.ascontiguousarray(
            xpad[sl].reshape(RT, P, KC, P).transpose(0, 3, 2, 1)
        ).reshape(RT, P, KC * P)
        in_maps.append({
            "xT": xt,
            "w1": w1_sw,
            "w2c": w2c_sw,
            "b1r": b1[None, :].astype(bf16),
            "bcr": bc_row.astype(bf16),
            "binv": binv_slot[sl][None, :].astype(bf16),
            "inv_s_t": np.ascontiguousarray(inv_s_slot[sl].reshape(RT, P).T),
            "inv_d_t": np.ascontiguousarray(inv_d_slot[sl].reshape(RT, P).T),
            "sc3": np.ascontiguousarray(sc3_slot[sl].reshape(RT, P).T),
            "pcol": plan["pcol"][c],
            "iota": iota128,
            "idxs": plan["idx_wrapped"][c],
        })
    return plan, in_maps


def kernel(features, edge_src, edge_dst, W1, b1, W2, b2, Wc, bc):
    features = np.asarray(features, np.float32)
    edge_src = np.asarray(edge_src)
    edge_dst = np.asarray(edge_dst)
    plan, in_maps = _prepare(features, edge_src, edge_dst,
                             np.asarray(W1, np.float32), np.asarray(b1, np.float32),
                             np.asarray(W2, np.float32), np.asarray(b2, np.float32),
                             np.asarray(Wc, np.float32), np.asarray(bc, np.float32))
    nc = _build_nc(plan)

    from concourse.bass_utils import run_bass_kernel_spmd
    res = run_bass_kernel_spmd(nc, in_maps, core_ids=list(range(NCORES)))

    out_slots = np.zeros((NP_PAD, N_CLS), np.float32)
    for c in range(NCORES):
        buf = res.results[c]["out"]
        out_slots[c * R:(c + 1) * R] = (
            buf.reshape(P, RT, N_CLS).transpose(1, 0, 2).reshape(R, N_CLS))
    perm = plan["perm"]
    return out_slots[perm[:N_NODES]]
